# revision 22
# baseline (speedup 1.0000x reference)
"""Dynamic Directional Attention on 8 trn2 NeuronCores (Bass/Tile), v3.

Problem: B=4, L=S=2048, H=8, E=64, f32.
  qt = tanh(q * 1/std_H(q) * dw) * dyn   (std over H=8, ddof=1; eps dropped)
  kt likewise; scores[b,h,l,s] = qt . kt  (contract E)
  tau[l] = sqrt(var_s(scores[l,:], ddof=1) + eps)
  A = softmax(scale * scores / tau);  out = A @ v  [B,L,H,E]

Sharding: 8 cores = 4 batches x 2 L-halves; per core q[1024, 512] plus full
k/v[2048, 512]. No collectives.

v3 design vs v2 (384us):
  - scores matmuls WITHOUT DoubleRow: DR streams at ~2 cyc/row on HW while
    plain fp8/bf16 streams 1 cyc/row at 2.4GHz. K=64 fits the array anyway.
    Layout: head-pair stacking [p=(hi*64+e), g, s/l], stationary tk [64,128],
    moving qs [64,512] -> out [128 s, 512 l] in PSUM.
  - single-pass fused preamble: each chunk loaded ONCE; per-chunk
    bn_stats (mean/M2 over H in one DVE pass) + gpsimd even/odd-combine +
    1-step Newton rsqrt on DVE (no ACT Sqrt table load); tanh natural-layout
    [p, h, 65] with fused ones column for k; PE transposes into pair layout.
  - Gram per head [64,64]+ksum col in one chained matmul set; ws/ssq/rs via
    block-diag pair matmuls; m broadcast via PE selector matmul (no DRAM
    bounce). ACT tables: tanh set, then natural_log_exp for m + main exp.
  - A@V unchanged (bf16, at-stationary, ones column for the denominator).
"""

import os
import sys

for _p in ("/opt/trn_rl_repo", "/root/.axon_site/_ro/trn_rl_repo"):
    if os.path.isdir(_p) and _p not in sys.path:
        sys.path.append(_p)

import numpy as np

import concourse.bass as bass
import concourse.mybir as mybir
import concourse.tile as tile
from concourse import bacc
from concourse.bass_utils import run_bass_kernel_spmd
from concourse.masks import make_identity

F32 = mybir.dt.float32
BF16 = mybir.dt.bfloat16
FP8 = mybir.dt.float8e4
I8 = mybir.dt.int8
I16 = mybir.dt.int16
I32 = mybir.dt.int32
AF = mybir.ActivationFunctionType
ALU = mybir.AluOpType

B, L, S, H, E = 4, 2048, 2048, 8, 64
LC = L // 2          # 1024 l-rows per core
D = H * E            # 512 free-dim columns (all 8 heads)
P = 128
NLT = LC // P        # 8 l-chunks (q)
NST = S // P         # 16 s-chunks (k, v)
HG = H // 2          # 4 head-pair groups; head = 2g + hi
EPS = 1e-6
SCALE = 1.0 / np.sqrt(E)
SQ7 = float(np.sqrt(H - 1))      # rstd = sqrt(7) * rsqrt(M2), folded in tanh
UNB_S = float(S) / float(S - 1)

QSF = 64.0           # fp8 pre-scale folded into m; exp uses scale=1/QSF
LOG2E = 1.4426950408889634
SCH_C = 0.04303      # Schraudolph mean-centering
SCH_A = 128.0 * LOG2E / QSF          # bf16 target: exp bits = x*A + B (int16)
SCH_B = 128.0 * (127.0 - SCH_C)
MAGIC = 0x5F3759DF   # rsqrt seed

# exp tiles per head routed to DVE (Schraudolph) instead of ACT
N_DVE_EXP = 6

_last_exec_time_ns = None


def _ensure_axon_hooks():
    """Provide antenv.axon_hooks (NTFF profiling hook) if the image lacks it."""
    try:
        import antenv.axon_hooks  # noqa: F401

        return
    except ImportError:
        pass
    import contextlib
    import ctypes
    import types

    try:
        import antenv
    except ImportError:
        return

    holder = {"h": None}
    mod = types.ModuleType("antenv.axon_hooks")
    mod.set_axon_ntff_profile_hook = lambda h: holder.__setitem__("h", h)
    mod.get_axon_ntff_profile_hook = lambda: holder["h"]
    sys.modules["antenv.axon_hooks"] = mod
    antenv.axon_hooks = mod

    so_path = "/opt/axon/libaxon_pjrt.so"
    if not os.path.exists(so_path):
        return
    try:
        lib = ctypes.CDLL(so_path)
    except OSError:
        return
    if not hasattr(lib, "axon_start_nrt_profile"):
        return
    lib.axon_start_nrt_profile.argtypes = [
        ctypes.POINTER(ctypes.c_int64),
        ctypes.c_size_t,
    ]
    lib.axon_start_nrt_profile.restype = ctypes.c_int64
    lib.axon_stop_nrt_profile.argtypes = [ctypes.c_char_p]
    lib.axon_stop_nrt_profile.restype = ctypes.c_int64

    @contextlib.contextmanager
    def _hook(output_dir, device_ids):
        import jax

        jax.devices()
        if device_ids:
            ids = (ctypes.c_int64 * len(device_ids))(*device_ids)
            rc = lib.axon_start_nrt_profile(ids, len(device_ids))
        else:
            rc = lib.axon_start_nrt_profile(None, 0)
        if rc != 0:
            raise RuntimeError(f"axon_start_nrt_profile rc={rc}")
        try:
            yield
        finally:
            n = lib.axon_stop_nrt_profile(str(output_dir).encode())
            print(f"profile: {n} file(s) written to {output_dir}", file=sys.stderr)

    holder["h"] = _hook


def _hbcast(ap_2d, nh=H):
    """View a [p, ne] AP as [p, nh, ne] with the head dim broadcast (step 0)."""
    return bass.AP(
        tensor=ap_2d.tensor,
        offset=ap_2d.offset,
        ap=[list(ap_2d.ap[0]), [0, nh], list(ap_2d.ap[1])],
    )


def build_nc():
    nc = bacc.Bacc("TRN2", target_bir_lowering=False, debug=False)
    q_d = nc.dram_tensor("q", [LC, D], F32, kind="ExternalInput")
    k_d = nc.dram_tensor("k", [S, D], F32, kind="ExternalInput")
    v_d = nc.dram_tensor("v", [S, D], F32, kind="ExternalInput")
    dw_d = nc.dram_tensor("dw", [1, 1], F32, kind="ExternalInput")
    dp_d = nc.dram_tensor("dp", [1, 1], F32, kind="ExternalInput")
    o_d = nc.dram_tensor("o", [LC, D], F32, kind="ExternalOutput")

    q_r = q_d.rearrange("(n p) d -> p n d", p=P)
    k_r = k_d.rearrange("(n p) d -> p n d", p=P)
    v_r = v_d.rearrange("(n p) d -> p n d", p=P)
    o_r = o_d.rearrange("(n p) d -> p n d", p=P)

    from contextlib import ExitStack

    with tile.TileContext(nc) as tc, ExitStack() as ctx:
        ek = ctx.enter_context
        sing = ek(tc.tile_pool(name="sing", bufs=1))
        pnat = ek(tc.tile_pool(name="nat", bufs=1))     # rotating loads
        pstat = ek(tc.tile_pool(name="stat", bufs=1))   # bns/M2/rstd/tmp
        pbig = ek(tc.tile_pool(name="big", bufs=1))     # persistent tensors

        # --- constants ---
        ident = sing.tile([P, P], BF16)
        make_identity(nc, ident)
        dw_t = sing.tile([P, 1], F32)
        nc.sync.dma_start(out=dw_t, in_=dw_d[:, :].to_broadcast([P, 1]))
        dp_t = sing.tile([P, 1], F32)
        nc.sync.dma_start(out=dp_t, in_=dp_d[:, :].to_broadcast([P, 1]))
        dwq = sing.tile([P, 1], F32)   # dw * sqrt(7): tanh scale
        nc.vector.tensor_scalar_mul(dwq, dw_t, SQ7)
        dp2 = sing.tile([P, 1], F32)
        nc.vector.tensor_mul(dp2, dp_t, dp_t)
        dp4u = sing.tile([P, 1], F32)  # dyn^4 * UNB_S  (ln scale)
        nc.vector.tensor_mul(dp4u, dp2, dp2)
        nc.vector.tensor_scalar_mul(dp4u, dp4u, float(UNB_S))
        c2 = sing.tile([P, 1], F32)    # QSF * SCALE * dyn^2
        nc.vector.tensor_scalar_mul(c2, dp2, float(QSF * SCALE))
        eps_t = sing.tile([P, 1], F32)
        nc.vector.memset(eps_t, EPS)

        # stationary selectors
        ones8g = sing.tile([P, HG, H], BF16)   # ssq col-sum per pair
        nc.vector.memset(ones8g, 0.0)
        for g in range(HG):
            for j in range(2):
                nc.vector.memset(ones8g[64 * j : 64 * (j + 1), g,
                                        2 * g + j : 2 * g + j + 1], 1.0)
        # selm (m row -> 64-block broadcast) = ones8g^T, built via PE transpose
        # (single-partition memsets at bases 1,2,... are illegal)
        selm = sing.tile([H, HG, P], BF16)
        with tc.tile_pool(name="selm_ps", bufs=1, space="PSUM") as psel:
            pselm = psel.tile([H, HG, P], BF16, tag="pselm",
                              padded_shape=[H, HG, 512])
            for g in range(HG):
                nc.tensor.transpose(pselm[:, g, :], ones8g[:, g, :], ident)
            nc.vector.tensor_copy(selm, pselm)

        # --- persistent tensors ---
        tkb = pbig.tile([P, HG, S], BF16, tag="tkb")
        tqb = pbig.tile([P, HG, LC], BF16, tag="tqb")
        qsb = pbig.tile([P, HG, LC], BF16, tag="qsb")
        va = pbig.tile([P, NST, H, 66], BF16, tag="va")
        osb = pbig.tile([P, NLT, D], F32, tag="osb")
        msb = pbig.tile([H, LC], BF16, tag="msb")
        x8 = pbig.tile([H, LC], F32, tag="x8")
        gsb = pbig.tile([64, H, 65], BF16, tag="gsb")
        gd2 = pbig.tile([P, HG, P], BF16, tag="gd2")
        ks8g = pbig.tile([P, HG, H], BF16, tag="ks8g")
        nc.vector.memset(gd2, 0.0)
        nc.vector.memset(ks8g, 0.0)

        ptg = ek(tc.tile_pool(name="tg", bufs=1))

        v_queue = list(range(NST))
        pair = [None, None]
        pend = []
        tgk = []

        def process_v(vi):
            vn = pnat.tile([P, D], F32, tag="nat", name=f"vn{vi}", bufs=6)
            nc.sync.dma_start(out=vn, in_=v_r[:, vi, :])
            nc.vector.tensor_copy(va[:, vi, :, 0:E],
                                  vn.rearrange("p (h e) -> p h e", h=H))

        def process_stats(kind, i, src):
            nat = pnat.tile([P, D], F32, tag="nat", name=f"nat_{kind}{i}",
                            bufs=6)
            nc.sync.dma_start(out=nat, in_=src[:, i, :])
            # stats over H in bf16 (2x DVE reduce rate):
            # M2 = ssq - ssum^2/H  (= sum (x-mu)^2)
            natb = pstat.tile([P, D], BF16, tag="natb", bufs=3)
            nc.vector.tensor_copy(natb, nat)
            sqb = pstat.tile([P, D], BF16, tag="sqb", bufs=3)
            nc.scalar.activation(sqb, natb, AF.Square, bias=0.0, scale=1.0)
            red = pstat.tile([P, 2, E], F32, tag="red", bufs=4)
            nc.vector.tensor_reduce(
                red[:, 0, :], natb.rearrange("p (h e) -> p e h", h=H),
                axis=mybir.AxisListType.X, op=ALU.add)
            nc.vector.tensor_reduce(
                red[:, 1, :], sqb.rearrange("p (h e) -> p e h", h=H),
                axis=mybir.AxisListType.X, op=ALU.add)
            j = len(pend)
            if j == 0:
                pair[0] = pstat.tile([P, 2, E], F32, tag="m2", bufs=2,
                                     name=f"m2_{kind}{i}")
                pair[1] = pstat.tile([P, 2, E], F32, tag="y", bufs=2,
                                     name=f"y_{kind}{i}")
            m2p, yp = pair
            nc.gpsimd.tensor_mul(m2p[:, j, :], red[:, 0, :], red[:, 0, :])
            nc.vector.scalar_tensor_tensor(m2p[:, j, :], m2p[:, j, :],
                                           -1.0 / H, red[:, 1, :],
                                           op0=ALU.mult, op1=ALU.add)
            pend.append((kind, i, nat, j))

        def flush_pair(ppre):
            # rstd' = rsqrt(M2): magic seed + 1 Newton step, per pair
            m2p, yp = pair
            nc.vector.tensor_scalar(out=yp.bitcast(I32),
                                    in0=m2p.bitcast(I32),
                                    scalar1=1, scalar2=None,
                                    op0=ALU.logical_shift_right)
            nc.vector.tensor_scalar(out=yp.bitcast(I32), in0=yp.bitcast(I32),
                                    scalar1=-1, scalar2=MAGIC,
                                    op0=ALU.mult, op1=ALU.add)
            a = pstat.tile([P, 2, E], F32, tag="nra", bufs=2)
            nc.vector.tensor_mul(a, yp, yp)
            nc.vector.scalar_tensor_tensor(a, a, -0.5, m2p,
                                           op0=ALU.mult, op1=ALU.mult)
            nc.vector.scalar_tensor_tensor(yp, a, 1.5, yp,
                                           op0=ALU.add, op1=ALU.mult)
            for kind, i, nat, j in pend:
                # tmp = nat * rstd' (broadcast over heads); tanh natural
                tmp = pstat.tile([P, D], F32, tag="tmp", bufs=3)
                nc.gpsimd.tensor_mul(tmp, nat, _hbcast(yp[:, j, :]))
                if kind == "k":
                    tg = ptg.tile([P, H, E], BF16, tag=f"tgk{i}",
                                  name=f"tgk{i}", bufs=1)
                    tgk.append(tg)
                else:
                    tg = ptg.tile([P, H, E], BF16, tag="tgq",
                                  name=f"tgq{i}", bufs=3)
                nc.scalar.activation(tg,
                                     tmp.rearrange("p (h e) -> p h e", h=H),
                                     AF.Tanh, bias=0.0, scale=dwq)
                # transpose into pair layout; copy on ACT
                pt = ppre.tile([P, HG, P], BF16, tag="pt",
                               padded_shape=[P, HG, 512], bufs=3)
                for g in range(HG):
                    nc.tensor.transpose(
                        pt[:, g, :], tg[:, 2 * g : 2 * g + 2, :], ident)
                dst = (tkb if kind == "k" else tqb)[:, :, P * i : P * (i + 1)]
                nc.scalar.copy(dst, pt)
            pend.clear()

        # ---------------- k phase ----------------
        with tc.tile_pool(name="prek_ps", bufs=1, space="PSUM") as ppre:
            G = ppre.tile([64, H, 65], F32, tag="G", padded_shape=[64, H, 96])
            for i in range(NST):
                process_stats("k", i, k_r)
                if len(pend) == 2:
                    flush_pair(ppre)
                if i % 2 == 1 and v_queue:
                    process_v(v_queue.pop(0))
            # Gram per head: G[e, e'] + ksum col 64. Chains per head are
            # sequential (concurrent chains in one PSUM bank corrupt each
            # other via the start-flag zero-region).
            ones1 = sing.tile([P, 1], BF16)
            nc.vector.memset(ones1, 1.0)
            for h in range(H):
                for i in range(NST):
                    nc.tensor.matmul(G[:, h, 0:E], tgk[i][:, h, :],
                                     tgk[i][:, h, :],
                                     start=(i == 0), stop=(i == NST - 1))
                for i in range(NST):
                    nc.tensor.matmul(G[:, h, E : E + 1], tgk[i][:, h, :],
                                     ones1, start=(i == 0),
                                     stop=(i == NST - 1))
            # G -> SBUF (scaled 1/S); build block-diag gd2 and ksum selectors
            nc.vector.tensor_scalar_mul(gsb, G, 1.0 / S)
            for g in range(HG):
                nc.vector.tensor_copy(gd2[0:64, g, 0:64], gsb[:, 2 * g, 0:64])
                nc.sync.dma_start(out=gd2[64:128, g, 64:128],
                                  in_=gsb[:, 2 * g + 1, 0:64])
                nc.vector.tensor_copy(ks8g[0:64, g, 2 * g : 2 * g + 1],
                                      gsb[:, 2 * g, 64:65])
                nc.sync.dma_start(out=ks8g[64:128, g, 2 * g + 1 : 2 * g + 2],
                                  in_=gsb[:, 2 * g + 1, 64:65])
            ln_c2 = sing.tile([H, 1], F32)
            nc.scalar.activation(ln_c2, c2[0:H, :], AF.Ln, bias=0.0, scale=1.0)

        # ---------------- q phase (ws/prod overlapped per l-half) ----------
        prods = {}
        with tc.tile_pool(name="preq_ps", bufs=1, space="PSUM") as ppre:
            for i in range(NLT):
                process_stats("q", i, q_r)
                if len(pend) == 2:
                    flush_pair(ppre)
                if v_queue:
                    process_v(v_queue.pop(0))
                if i % 4 == 3:
                    hf = i // 4
                    sl = slice(512 * hf, 512 * (hf + 1))
                    for g in range(HG):
                        ws = ppre.tile([P, 512], F32, tag="ws", bufs=2,
                                       name=f"ws{g}_{hf}")
                        nc.tensor.matmul(ws, gd2[:, g, :], tqb[:, g, sl],
                                         start=True, stop=True)
                        prod = pstat.tile([P, 512], BF16, tag="prod", bufs=8,
                                          name=f"prod{g}_{hf}")
                        nc.vector.tensor_mul(prod, ws, tqb[:, g, sl])
                        prods[(g, hf)] = prod
            while v_queue:
                process_v(v_queue.pop(0))
            # ones column for the A@V denominator
            nc.vector.memset(
                va.rearrange("p n h c -> p (n h) c")[:, :, E : E + 1], 1.0)

        # ============ score row stats -> m -> qs ============
        with tc.tile_pool(name="st_ps", bufs=1, space="PSUM") as pst:
            ssq_ps = pst.tile([H, LC], F32, tag="ssq")
            rs_ps = pst.tile([H, LC], F32, tag="rs")
            for hf in range(2):
                sl = slice(512 * hf, 512 * (hf + 1))
                for g in range(HG):
                    nc.tensor.matmul(ssq_ps[:, sl], ones8g[:, g, :],
                                     prods[(g, hf)],
                                     start=(g == 0), stop=(g == HG - 1))
                for g in range(HG):
                    nc.tensor.matmul(rs_ps[:, sl], ks8g[:, g, :],
                                     tqb[:, g, sl],
                                     start=(g == 0), stop=(g == HG - 1))
            # m = c2 / tau', tau'^2 = dyn^4*UNB*(ssq - rs^2) + eps
            nc.scalar.activation(x8, rs_ps, AF.Square, bias=0.0, scale=1.0)
            nc.vector.tensor_sub(x8, ssq_ps, x8)
            nc.scalar.activation(x8, x8, AF.Ln, bias=eps_t[0:H, :],
                                 scale=dp4u[0:H, :])
            nc.scalar.activation(msb, x8, AF.Exp, bias=ln_c2, scale=-0.5)
            # qs = tq * m (broadcast m rows across 64-blocks via PE selector)
            for g in range(HG):
                mb = pst.tile([P, LC], F32, tag="mb", bufs=2, name=f"mb{g}")
                for n0 in range(0, LC, 512):
                    nc.tensor.matmul(mb[:, n0 : n0 + 512], selm[:, g, :],
                                     msb[:, n0 : n0 + 512],
                                     start=True, stop=True)
                nc.vector.tensor_mul(qsb[:, g, :], tqb[:, g, :], mb)

        # ============ main loop ============
        with tc.tile_pool(name="mm_ps", bufs=1, space="PSUM") as pmm, \
             tc.tile_pool(name="at_pool", bufs=1) as pat:

            def emit_exp(at, st_ps, kk):
                if kk >= NST - N_DVE_EXP:
                    nc.vector.tensor_scalar(
                        out=at[:, kk, :].bitcast(I16), in0=st_ps,
                        scalar1=SCH_A, scalar2=SCH_B,
                        op0=ALU.mult, op1=ALU.add)
                else:
                    nc.scalar.activation(at[:, kk, :], st_ps, AF.Exp,
                                         bias=0.0, scale=1.0 / QSF)

            def emit_av_lt(h, at, lt, po):
                for kx in range(NST):
                    nc.tensor.matmul(
                        po[:, lt, 0 : E + 1],
                        at[:, kx, lt * P : (lt + 1) * P],
                        va[:, kx, h, 0 : E + 1],
                        start=(kx == 0), stop=(kx == NST - 1))

            def emit_epilogue(h, po):
                rc = pstat.tile([P, NLT, 1], F32, tag="rc", bufs=2,
                                name=f"rc{h}")
                nc.vector.reciprocal(rc, po[:, :, E : E + 1])
                for lt in range(NLT):
                    nc.vector.tensor_scalar_mul(
                        osb[:, lt, E * h : E * (h + 1)], po[:, lt, 0:E],
                        rc[:, lt, :])
                    if h == H - 1:
                        nc.sync.dma_start(out=o_r[:, lt, :], in_=osb[:, lt, :])

            prev = None
            for h in range(H):
                g, hi = h // 2, h % 2
                tks = tkb[64 * hi : 64 * (hi + 1), g, :]
                qss = qsb[64 * hi : 64 * (hi + 1), g, :]
                at = pat.tile([P, NST, LC], BF16, tag="at", bufs=2,
                              name=f"at{h}")
                po_h = pmm.tile([P, NLT, E + 1], F32, tag="po", bufs=2,
                                name=f"po{h}", padded_shape=[P, NLT, P])
                for kk in range(NST):
                    st_ps = pmm.tile([P, LC], F32, tag="stp", bufs=2,
                                     name=f"st{h}_{kk}")
                    for n0 in range(0, LC, 512):
                        nc.tensor.matmul(
                            st_ps[:, n0 : n0 + 512],
                            tks[:, P * kk : P * (kk + 1)],
                            qss[:, n0 : n0 + 512],
                            start=True, stop=True)
                    emit_exp(at, st_ps, kk)
                    if prev is not None and kk % 2 == 1:
                        ph, pat_t, ppo = prev
                        emit_av_lt(ph, pat_t, kk // 2, ppo)
                        if kk == NST - 1:
                            emit_epilogue(ph, ppo)
                prev = (h, at, po_h)
            ph, pat_t, ppo = prev
            for lt in range(NLT):
                emit_av_lt(ph, pat_t, lt, ppo)
            emit_epilogue(ph, ppo)

    return nc


_nc_cache = None


def kernel(queries, keys, values, attn_mask=None, directional_weights=None,
           dynamic_param=None, **_unused):
    global _nc_cache, _last_exec_time_ns
    q = np.asarray(queries, dtype=np.float32)
    k = np.asarray(keys, dtype=np.float32)
    v = np.asarray(values, dtype=np.float32)
    if directional_weights is None:
        dw = np.ones((1, 1), dtype=np.float32)
    else:
        dw = np.asarray(directional_weights, dtype=np.float32).reshape(1, 1)
    if dynamic_param is None:
        dp = np.ones((1, 1), dtype=np.float32)
    else:
        dp = np.asarray(dynamic_param, dtype=np.float32).reshape(1, 1)

    if _nc_cache is None:
        nc = build_nc()
        nc.finalize()
        _nc_cache = nc
    nc = _nc_cache

    in_maps = []
    for c in range(8):
        b, lh = c // 2, c % 2
        in_maps.append({
            "q": np.ascontiguousarray(q[b, lh * LC : (lh + 1) * LC]).reshape(LC, D),
            "k": np.ascontiguousarray(k[b]).reshape(S, D),
            "v": np.ascontiguousarray(v[b]).reshape(S, D),
            "dw": dw, "dp": dp,
        })

    tracing = bool(os.environ.get("BASS_TRACE"))
    if tracing:
        _ensure_axon_hooks()
        import concourse.bass_utils as _bu

        _orig_upload = _bu.upload_artifacts
        _bu.upload_artifacts = lambda d: d
        try:
            res = run_bass_kernel_spmd(nc, in_maps, core_ids=list(range(8)))
        except Exception as e:
            print(f"traced run failed ({e!r}); retrying untraced", file=sys.stderr)
            os.environ["BASS_NEVER_TRACE"] = "1"
            try:
                res = run_bass_kernel_spmd(nc, in_maps, core_ids=list(range(8)))
            finally:
                os.environ.pop("BASS_NEVER_TRACE", None)
        finally:
            _bu.upload_artifacts = _orig_upload
    else:
        res = run_bass_kernel_spmd(nc, in_maps, core_ids=list(range(8)))
    _last_exec_time_ns = res.exec_time_ns

    out = np.empty((B, L, H, E), dtype=np.float32)
    for c in range(8):
        b, lh = c // 2, c % 2
        out[b, lh * LC : (lh + 1) * LC] = res.results[c]["o"].reshape(LC, H, E)
    return out
